# revision 1
# baseline (speedup 1.0000x reference)
"""Trainium2 Bass kernel for nn_Block_27127013442041 (GNN message passing block).

Self-contained: takes FULL inputs, shards per-core internally (node blocks +
src-owned edges), runs one SPMD Bass program on 8 NeuronCores with a mid-kernel
AllGather, returns FULL outputs.
"""
import sys as _sys
_sys.path.insert(0, "/root/problem/work")  # replaced by inline lib below if absent
import numpy as np

_KLIB_READY = False


def _ensure_lib():
    global _KLIB_READY, KL
    if _KLIB_READY:
        return
    import kernel_lib as KL_mod
    globals()['KL'] = KL_mod
    _KLIB_READY = True


_PROG_CACHE = {}


def kernel(**inputs):
    _ensure_lib()
    meta, com, cores = KL.host_prep(**inputs)
    key = (meta['pblk'], meta['prblk'])
    if key not in _PROG_CACHE:
        prog = KL.Prog(meta, com, cores[0], phase="ABC", fake_coll=False)
        _PROG_CACHE[key] = prog.build()
    nc = _PROG_CACHE[key]
    in_maps = [{**com, **c} for c in cores]
    res = KL.run_bass_kernel_spmd(nc, in_maps, core_ids=list(range(KL.NCORES)))
    N = KL.N
    xa = res.results[0]["oxa"].T.copy()
    xv = res.results[0]["oxv"].T.copy()
    for c in range(KL.NCORES):
        sl = slice(1024 * c, 1024 * (c + 1))
        psi = res.results[c]["opsi"]
        xa[sl] += psi[:, :64]
        xv[sl] += psi[:, 64:]
    out_a = xa[None].astype(np.float32)
    out_v = np.ascontiguousarray(xv.reshape(N, 3, 32).transpose(0, 2, 1))[None].astype(np.float32)
    return out_a, out_v
